# revision 1
# baseline (speedup 1.0000x reference)
"""AbsDiagNet Trainium2 kernel.

Computes:
    xW = einsum('sbi,hi->sbh', X, W_ih)
    h_{t+1} = |xW_t + HH * h_t|   (h_0 = 0, scan over S)
    out = h_S @ W_ho.T + b_ho

Strategy: data-parallel over batch B across 8 NeuronCores (16 rows each).
Per core, the big GEMM runs on TensorE in tiered mixed precision: h-tiles
0..7 take the first 256 I-columns in fp8-e4m3 DoubleRow mode (K=256 per
instruction at 2x rate) + 768 in bf16; h-tiles 8..15 take 512 fp8 columns
(2 DoubleRows) + 512 bf16. The output projection averages over all H, so
the wide-fp8 tier's extra quantization noise stays under the accuracy
budget while cutting another matmul per group. Operands carry power-of-two scales (X*32,
W*2048) so the fp8/bf16 partial products accumulate at a common scale in
PSUM; the 2^-16 descale is folded into the output projection, which is
exact because the |.| recurrence is positively homogeneous. The 512-step
recurrence is ONE custom DVE instruction per (h_tile, b): an inclusive
scan with ALU op ABSOLUTE_DIFF (state = |state - (-xW_t)| = |state +
xW_t|) -- W is negated on the host. The final projection is a small f32r
matmul, bias added on VectorE.
"""
import sys

sys.path.insert(0, "/opt/trn_rl_repo")

import numpy as np
import ml_dtypes

import concourse.bass as bass  # noqa: F401
import concourse.tile as tile
from concourse import mybir, bacc
from concourse.bass_utils import run_bass_kernel_spmd

S, B, I, H, O = 512, 128, 1024, 2048, 1024
NCORES = 8
BL = B // NCORES          # 16 batch rows per core
HT = H // 128             # 16 hidden tiles
KF8 = 256                 # fp8 I-columns for h-tiles 0..7 (1 DoubleRow)
KF8A = 512                # fp8 I-columns for h-tiles 8..15 (2 DoubleRows)
HTA = 8                   # first h-tile of the wide-fp8 tier
JT = (I - KF8) // 128     # 6 bf16 contraction tiles (narrow tier)
JTA = (I - KF8A) // 128   # 4 bf16 contraction tiles (wide tier)
SX = 32.0                 # X scale (power of 2; exact)
SW = 2048.0               # W scale (power of 2; exact)
XT_BUFS = 6
PS_BUFS = 6
SCAN_BUFS = 4

F32 = mybir.dt.float32
F32R = mybir.dt.float32r
BF16 = mybir.dt.bfloat16
E4M3 = mybir.dt.float8e4


# --- custom DVE op: inclusive scan  state = |state - x|  -------------------
def _make_absdiff_scan_op():
    import concourse.dve_ops as dve_ops
    from concourse.dve_spec import Spec, Src0, Zero, AluOp, scan, lower
    from concourse.dve_uop import DveOpSpec

    NAME = "ABS_DIFF_SCAN_ANT"
    for op in dve_ops.OPS:
        if op.name == NAME:
            return op

    def ref(in0, in1, s0, s1, imm2):
        out = np.zeros_like(in0, dtype=np.float32)
        st = np.zeros(in0.shape[:-1], np.float32)
        for t in range(in0.shape[-1]):
            st = np.abs(st - in0[..., t])
            out[..., t] = st
        return out

    spec = Spec(body=scan(AluOp.ABSOLUTE_DIFF, Src0, init=Zero), reference=ref)
    row = dve_ops._CUSTOM_DVE_ROW_BASE + len(dve_ops.OPS)
    shas = {}
    for ver in ("v3", "v4"):
        try:
            shas[ver] = DveOpSpec(
                name=NAME, opcode=row, uops=lower(spec, ver=ver), rd1_en=False
            ).sha(ver)
        except Exception:
            pass
    op = dve_ops.DveOp(NAME, spec, subdim=False, uops_sha=shas)
    dve_ops.OPS.append(op)
    dve_ops.CUSTOM_DVE_SPECS[NAME] = spec
    dve_ops._SUB_OPCODE_FOR_NAME[NAME] = row
    return op


_ABS_DIFF_SCAN = _make_absdiff_scan_op()
_NC_CACHE = {}


def _build_nc():
    if "nc" in _NC_CACHE:
        return _NC_CACHE["nc"]
    nc = bacc.Bacc("TRN2", target_bir_lowering=False, debug=False,
                   num_devices=NCORES)
    xt = nc.declare_dram_parameter("xt", [128, BL * JT * S], BF16, isOutput=False)
    xt8 = nc.declare_dram_parameter("xt8", [128, BL, 4, S], E4M3, isOutput=False)
    wt = nc.declare_dram_parameter("wt", [128, (HTA * JT + (HT - HTA) * JTA) * 128], BF16, isOutput=False)
    w8 = nc.declare_dram_parameter("w8", [128, HTA * 2 + (HT - HTA) * 4, 128], E4M3, isOutput=False)
    who = nc.declare_dram_parameter("who", [128, HT * O], F32R, isOutput=False)
    bias = nc.declare_dram_parameter("bias", [BL, O], F32, isOutput=False)
    out = nc.declare_dram_parameter("out", [BL, O], F32, isOutput=True)

    with tile.TileContext(nc) as tc:
        with (
            tc.tile_pool(name="const", bufs=1) as cpool,
            tc.tile_pool(name="xt", bufs=XT_BUFS) as xpool,
            tc.tile_pool(name="xt8", bufs=XT_BUFS) as x8pool,
            tc.tile_pool(name="scan", bufs=SCAN_BUFS) as spool,
            tc.tile_pool(name="ps", bufs=PS_BUFS, space="PSUM") as pspool,
            tc.tile_pool(name="ps2", bufs=2, space="PSUM") as ps2pool,
        ):
            # weights arrive in four chunks (deps are tile-granular): the
            # first matmul group only waits for the first chunk; later chunks
            # stream in behind while earlier h-tiles compute
            WSPLITS = [(0, 2), (2, 4), (4, 8), (8, 16)]

            def n_dr(ht):      # fp8 DoubleRow matmuls per h-tile
                return 2 if ht >= HTA else 1

            def n_bf(ht):      # bf16 chunks per h-tile
                return JTA if ht >= HTA else JT

            def j_off(ht):     # first bf16 chunk index within the xt layout
                return JT - n_bf(ht)

            wt_tiles_s = []
            w8_tiles_s = []
            for i, (h0, h1) in enumerate(WSPLITS):
                nbf = n_bf(h0)
                wt_tiles_s.append(cpool.tile(
                    [128, (h1 - h0) * nbf * 128], BF16, tag=f"wtp{i}",
                    name=f"wtp{i}"))
                w8_tiles_s.append(cpool.tile(
                    [128, (h1 - h0) * 2 * n_dr(h0), 128], E4M3, tag=f"w8p{i}",
                    name=f"w8p{i}"))
            who_sb = cpool.tile([128, HT * O], F32R, tag="who")
            bias_sb = cpool.tile([BL, O], F32, tag="bias")
            hfinal = cpool.tile([128, HT * BL], F32R, tag="hfinal")

            def wt_slice(ht, j):
                for (h0, h1), t in zip(WSPLITS, wt_tiles_s):
                    if h0 <= ht < h1:
                        c = ((ht - h0) * n_bf(ht) + j) * 128
                        return t[:, c:c + 128]
                raise AssertionError

            def w8_slice(ht, dr=0):
                for (h0, h1), t in zip(WSPLITS, w8_tiles_s):
                    if h0 <= ht < h1:
                        o = 2 * (n_dr(ht) * (ht - h0) + dr)
                        return t[:, o:o + 2, :]
                raise AssertionError

            from bass_rust import add_dep_helper

            xt_tiles = [None] * BL
            xt8_tiles = [None] * BL
            # serialize the startup transfers: concurrent queues would split
            # bandwidth and delay the critical first chunks; the first matmul
            # (fp8 DoubleRow) only needs xt8[0] + w8 part 0
            xt8_tiles[0] = x8pool.tile([128, 4, S], E4M3, tag="x8b", name="x8b0")
            prev_dma = nc.sync.dma_start(xt8_tiles[0][:, 0:2, :], xt8[:, 0, 0:2])

            def chain(d, sync=False):
                nonlocal prev_dma
                add_dep_helper(d.ins, prev_dma.ins, sync=sync, reason="dma order")
                prev_dma = d

            # w8 part 0 rides a second queue concurrently with xt8[0]: both
            # gate the very first (fp8 DoubleRow) matmul and are small
            d_w8p0 = nc.sync.dma_start(w8_tiles_s[0][:],
                                       w8[:, 2 * WSPLITS[0][0]:2 * WSPLITS[0][1], :])
            # b=0's bf16 X and ht=0's weights land as split tiles so the
            # first group can start on j=0 while the rest streams in (deps
            # are tile-granular)
            xt0_parts = [
                cpool.tile([128, S], BF16, tag="xt0a", name="xt0a"),
                cpool.tile([128, S], BF16, tag="xt0b", name="xt0b"),
                cpool.tile([128, (JT - 2) * S], BF16, tag="xt0c", name="xt0c"),
            ]
            wt0_parts = [
                cpool.tile([128, 2 * 128], BF16, tag="wt0a", name="wt0a"),
                cpool.tile([128, (JT - 2) * 128], BF16, tag="wt0b", name="wt0b"),
            ]
            d_wt0a = nc.sync.dma_start(wt0_parts[0][:], wt[:, 0:2 * 128])
            add_dep_helper(d_wt0a.ins, d_w8p0.ins, sync=False, reason="dma order")
            chain(nc.sync.dma_start(xt0_parts[0][:], xt[:, 0:S]))
            # w8 part 1 hoisted early: it unlocks the ht=2..3 partial fp8
            # matmuls that keep the PE busy while b=0's bulk X streams in
            chain(nc.sync.dma_start(
                w8_tiles_s[1][:], w8[:, 2 * WSPLITS[1][0]:2 * WSPLITS[1][1], :]))
            chain(nc.sync.dma_start(xt0_parts[1][:], xt[:, S:2 * S]))
            chain(nc.sync.dma_start(wt0_parts[1][:], wt[:, 2 * 128:JT * 128]))
            chain(nc.sync.dma_start(xt0_parts[2][:], xt[:, 2 * S:JT * S]))
            chain(nc.sync.dma_start(
                wt_tiles_s[0][:],
                wt[:, WSPLITS[0][0] * JT * 128:WSPLITS[0][1] * JT * 128]))

            def xt_chunk(b, j):
                if b == 0:
                    if j < 2:
                        return xt0_parts[j][:]
                    return xt0_parts[2][:, (j - 2) * S:(j - 1) * S]
                return xt_tiles[b][:, j * S:(j + 1) * S]

            def wt_chunk(b, ht, j):
                if b == 0 and ht == 0:
                    if j < 2:
                        return wt0_parts[0][:, j * 128:(j + 1) * 128]
                    return wt0_parts[1][:, (j - 2) * 128:(j - 1) * 128]
                return wt_slice(ht, j)
            def w8_doff(h):
                return 2 * min(h, HTA) + 4 * max(0, h - HTA)

            def wt_doff(h):
                return (JT * min(h, HTA) + JTA * max(0, h - HTA)) * 128

            for i, (h0, h1) in enumerate(WSPLITS):
                if i == 0:
                    continue
                if i != 1:
                    chain(nc.sync.dma_start(w8_tiles_s[i][:],
                                            w8[:, w8_doff(h0):w8_doff(h1), :]))
                chain(nc.sync.dma_start(
                    wt_tiles_s[i][:], wt[:, wt_doff(h0):wt_doff(h1)]))

            # prefetch b=1 right behind the startup chain, then the epilogue
            # constants (order matters: all of these share the 16 SDMA engines)
            chain(nc.sync.dma_start(xt8_tiles[0][:, 2:4, :], xt8[:, 0, 2:4]))
            xt8_tiles[1] = x8pool.tile([128, 4, S], E4M3, tag="x8b", name="x8b1")
            chain(nc.sync.dma_start(xt8_tiles[1][:], xt8[:, 1]))
            xt_tiles[1] = xpool.tile([128, JT * S], BF16, tag="xtb", name="xtb1")
            chain(nc.sync.dma_start(xt_tiles[1][:], xt[:, JT * S:2 * JT * S]))
            dwho = nc.gpsimd.dma_start(who_sb[:], who[:])
            add_dep_helper(dwho.ins, prev_dma.ins, sync=True,
                           reason="defer epilogue consts")
            dbias = nc.gpsimd.dma_start(bias_sb[:], bias[:])
            add_dep_helper(dbias.ins, dwho.ins, sync=True,
                           reason="defer epilogue consts")

            outsb = [cpool.tile([BL, 512], F32, tag=f"outsb{oc}",
                                name=f"outsb{oc}") for oc in range(O // 512)]
            ps2_tiles = [None, None]

            def emit_final(ht, ocs=(0, 1)):
                for oc in ocs:
                    if ps2_tiles[oc] is None:
                        ps2_tiles[oc] = ps2pool.tile(
                            [BL, 512], F32, tag="ps2", name=f"ps2_{oc}")
                    nc.tensor.matmul(
                        ps2_tiles[oc][:],
                        hfinal[:, ht * BL:(ht + 1) * BL],
                        who_sb[:, ht * O + oc * 512: ht * O + oc * 512 + 512],
                        start=(ht == 0),
                        stop=(ht == HT - 1),
                    )

            def ensure_xt(b):
                if b > 0 and xt_tiles[b] is None:
                    xt8_tiles[b] = x8pool.tile([128, 4, S], E4M3, tag="x8b",
                                               name=f"x8b{b}")
                    nc.sync.dma_start(xt8_tiles[b][:], xt8[:, b])
                    xt_tiles[b] = xpool.tile([128, JT * S], BF16, tag="xtb",
                                             name=f"xtb{b}")
                    nc.sync.dma_start(
                        xt_tiles[b][:], xt[:, b * JT * S:(b + 1) * JT * S])

            def group_tail(b, ht, ps):
                so = spool.tile([128, S], F32, tag="so")
                nc.vector._custom_dve(_ABS_DIFF_SCAN, out=so[:], in0=ps[:])
                c = ht * BL + b
                nc.scalar.copy(hfinal[:, c:c + 1], so[:, S - 1:S])

            # b=0 catch-up phase: while b=0's bulk X streams in, run the
            # matmuls whose operands arrive first (fp8 + early bf16 chunks)
            # as partial accumulation groups, then complete them in order
            PARTIAL = [(0, 2), (1, 0), (2, 0), (3, 0)]
            pre = []
            for ht, nj in PARTIAL:
                ps = pspool.tile([128, S], F32, tag="ps")
                nc.tensor.matmul(
                    ps[:], w8_slice(ht), xt8_tiles[0][:, 0:2, :],
                    start=True, stop=False,
                    perf_mode=mybir.MatmulPerfMode.DoubleRow,
                )
                for j in range(nj):
                    nc.tensor.matmul(
                        ps[:], wt_chunk(0, ht, j), xt_chunk(0, j),
                        start=False, stop=False,
                    )
                pre.append((ht, nj, ps))
            for ht, nj, ps in pre:
                for j in range(nj, JT):
                    nc.tensor.matmul(
                        ps[:], wt_chunk(0, ht, j), xt_chunk(0, j),
                        start=False, stop=(j == JT - 1),
                    )
                group_tail(0, ht, ps)

            # steady state: remaining (b, h-tile) groups flattened and
            # processed in batches of 6 (= PSUM pool size) with all six fp8
            # DoubleRow matmuls adjacent: the PE pays a small penalty per
            # fp8<->bf16 mode switch, so batching the fp8 work cuts the
            # switch count from 2/group to 2/batch
            groups = [(0, ht) for ht in range(len(PARTIAL), HT)]
            for b in range(1, BL):
                groups += [(b, ht) for ht in range(HT)]
            for k in range(0, len(groups), PS_BUFS):
                batch = groups[k:k + PS_BUFS]
                pss = []
                for b, ht in batch:
                    ensure_xt(b)
                    ps = pspool.tile([128, S], F32, tag="ps")
                    for dr in range(n_dr(ht)):
                        nc.tensor.matmul(
                            ps[:], w8_slice(ht, dr),
                            xt8_tiles[b][:, 2 * dr:2 * dr + 2, :],
                            start=(dr == 0), stop=False,
                            perf_mode=mybir.MatmulPerfMode.DoubleRow,
                        )
                    pss.append(ps)
                for (b, ht), ps in zip(batch, pss):
                    nbf, jo = n_bf(ht), j_off(ht)
                    for j in range(nbf):
                        nc.tensor.matmul(
                            ps[:],
                            wt_chunk(b, ht, j),
                            xt_chunk(b, jo + j),
                            start=False,
                            stop=(j == nbf - 1),
                        )
                    group_tail(b, ht, ps)
                # epilogue matmuls for b=15's h-tiles, one batch behind so
                # their scan+copy dependencies have cleared; emitted as one
                # burst per batch to avoid extra f32r<->bf16 mode switches
                for b, ht in batch:
                    if b == BL - 1 and ht > 0:
                        emit_final(ht - 1)

            # last h-tile: finish each 512-column half independently so the
            # first half's bias-add + store overlap the second half's matmul
            for oc in range(O // 512):
                emit_final(HT - 1, ocs=(oc,))
                nc.vector.tensor_tensor(
                    outsb[oc][:], ps2_tiles[oc][:],
                    bias_sb[:, oc * 512:(oc + 1) * 512], op=mybir.AluOpType.add,
                )
                nc.sync.dma_start(out[:, oc * 512:(oc + 1) * 512], outsb[oc][:])
    nc.finalize()
    _NC_CACHE["nc"] = nc
    return nc


def _prep_inputs(X, W_ih, HH, W_ho, b_ho):
    """Host-side sharding + relayout + quantization.

    Returns list of per-core input maps."""
    X = np.asarray(X, dtype=np.float32)
    W_ih = np.asarray(W_ih, dtype=np.float32)
    HH = np.asarray(HH, dtype=np.float32)
    W_ho = np.asarray(W_ho, dtype=np.float32)
    b_ho = np.asarray(b_ho, dtype=np.float32)

    # Fold sign(HH) into W_ih rows: |xw + HH*h| = |sgn*xw + |HH|*h| for h>=0.
    if not np.all(HH == 1.0):
        sgn = np.where(HH < 0, -1.0, 1.0).astype(np.float32)
        W_ih = W_ih * sgn[:, None]
        HH = np.abs(HH)
        if not np.allclose(HH, 1.0):
            raise NotImplementedError(
                "general |HH| != 1 recurrence not implemented in this kernel"
            )

    Wn = -W_ih * SW  # negated (scan computes |h - (-xw)|) and pre-scaled
    # per h-tile: tiles < HTA take KF8 fp8 columns (2 slots) + JT bf16
    # chunks; tiles >= HTA take KF8A fp8 columns (4 slots) + JTA bf16
    wt_parts, w8_parts = [], []
    for ht in range(HT):
        kf = KF8 if ht < HTA else KF8A
        blk = Wn[ht * 128:(ht + 1) * 128]          # [128(hh), I]
        wt_parts.append(np.ascontiguousarray(
            blk[:, kf:].reshape(128, (I - kf) // 128, 128).transpose(2, 1, 0)
        ).reshape(128, -1))                        # [128(p), nbf*128]
        w8_parts.append(np.ascontiguousarray(
            blk[:, :kf].reshape(128, kf // 128, 128).transpose(2, 1, 0)
        ))                                         # [128(p), nslots, 128(hh)]
    wt = np.concatenate(wt_parts, axis=1).astype(ml_dtypes.bfloat16)
    w8 = np.concatenate(w8_parts, axis=1).astype(ml_dtypes.float8_e4m3)
    # who[p, ht, o] = W_ho[o, ht*128+p] / (SX*SW)  (descale folded in)
    who = np.ascontiguousarray(
        (W_ho / (SX * SW)).reshape(O, HT, 128).transpose(2, 1, 0)
    ).reshape(128, -1)
    bias = np.ascontiguousarray(np.broadcast_to(b_ho, (BL, O)))

    in_maps = []
    for c in range(NCORES):
        Xc = X[:, c * BL:(c + 1) * BL, :] * SX  # [S, BL, I], pre-scaled
        # xt[p, b, j, s] = Xc[s, b, KF8 + j*128 + p]
        xt = np.ascontiguousarray(
            Xc[:, :, KF8:].reshape(S, BL, JT, 128).transpose(3, 1, 2, 0)
        ).reshape(128, -1).astype(ml_dtypes.bfloat16)
        # xt8[p, b, cc, s] = Xc[s, b, cc*128+p]  (4 slots: k 0..KF8A)
        xt8 = np.ascontiguousarray(
            Xc[:, :, :KF8A].reshape(S, BL, 4, 128).transpose(3, 1, 2, 0)
        ).astype(ml_dtypes.float8_e4m3)
        in_maps.append({"xt": xt, "xt8": xt8, "wt": wt, "w8": w8,
                        "who": who, "bias": bias})
    return in_maps


def _run(in_maps, **kwargs):
    nc = _build_nc()
    return run_bass_kernel_spmd(nc, in_maps, core_ids=list(range(NCORES)), **kwargs)


def kernel(X, W_ih, HH, W_ho, b_ho):
    in_maps = _prep_inputs(X, W_ih, HH, W_ho, b_ho)
    res = _run(in_maps)
    return np.concatenate([res.results[c]["out"] for c in range(NCORES)], axis=0)



# revision 2
# speedup vs baseline: 1.5830x; 1.5830x over previous
"""AbsDiagNet Trainium2 kernel — all-fp8 DoubleRow GEMM.

Computes:
    xW = einsum('sbi,hi->sbh', X, W_ih)
    h_{t+1} = |xW_t + HH * h_t|   (h_0 = 0, scan over S)
    out = h_S @ W_ho.T + b_ho

Strategy: data-parallel over batch B across 8 NeuronCores (16 rows each).
The big GEMM runs entirely in fp8-e4m3 DoubleRow mode (K=256 per
instruction at 2x rate): per (batch-row, h-tile) group it is 4 DR matmuls
accumulating ps[128, 512] over the full I=1024 contraction.

The 2e-2 accuracy gate is met at full-fp8 via two host-side quantization
refinements (the scan behaves like a running sum once h grows, so quant
errors accumulate coherently over the 512 timesteps):
  1. X is quantized with first-order noise shaping (error feedback) along
     t, which bounds every time-window sum of X quant error to ~1 ulp
     instead of sqrt(512) ulps.
  2. W's per-element rounding direction is optimized per core (GPTQ-style)
     so its quantization error is near-orthogonal to the per-batch X
     prefix sums at 8 sqrt-spaced time windows — cancelling the coherent
     W-error term in the scan.
Operands carry power-of-two scales (X*32, W*2048); the 2^-16 descale is
folded into the output projection, exact because the |.| recurrence is
positively homogeneous. The 512-step recurrence is ONE custom DVE
instruction per (h_tile, b): an inclusive scan with ALU op ABSOLUTE_DIFF
(state = |state - (-xW_t)| = |state + xW_t|) -- W is negated on the host.
The final projection is a small f32r matmul, bias added on VectorE.
"""
import sys

sys.path.insert(0, "/opt/trn_rl_repo")

import numpy as np
import ml_dtypes

import concourse.bass as bass  # noqa: F401
import concourse.tile as tile
from concourse import mybir, bacc
from concourse.bass_utils import run_bass_kernel_spmd

S, B, I, H, O = 512, 128, 1024, 2048, 1024
NCORES = 8
BL = B // NCORES          # 16 batch rows per core
HT = H // 128             # 16 hidden tiles
NSL = I // 128            # 8 fp8 pair-slots per group (K=128 each)
KDR = NSL // 2            # 4 DoubleRow matmuls per group (K=256 each)
SX = 32.0                 # X scale (power of 2; exact)
SW = 2048.0               # W scale (power of 2; exact)
XT_BUFS = 6
PS_BUFS = 6
SCAN_BUFS = 4
NWIN = 8                  # W-fix window count
FLIP_ITERS = 150
FLIP_LAM = 0.3

F32 = mybir.dt.float32
F32R = mybir.dt.float32r
E4M3 = mybir.dt.float8e4

E4NP = ml_dtypes.float8_e4m3  # IEEE-style: max 240 matches TRN FP8_EXP4


# --- custom DVE op: inclusive scan  state = |state - x|  -------------------
def _make_absdiff_scan_op():
    import concourse.dve_ops as dve_ops
    from concourse.dve_spec import Spec, Src0, Zero, AluOp, scan, lower
    from concourse.dve_uop import DveOpSpec

    NAME = "ABS_DIFF_SCAN_ANT"
    for op in dve_ops.OPS:
        if op.name == NAME:
            return op

    def ref(in0, in1, s0, s1, imm2):
        out = np.zeros_like(in0, dtype=np.float32)
        st = np.zeros(in0.shape[:-1], np.float32)
        for t in range(in0.shape[-1]):
            st = np.abs(st - in0[..., t])
            out[..., t] = st
        return out

    spec = Spec(body=scan(AluOp.ABSOLUTE_DIFF, Src0, init=Zero), reference=ref)
    row = dve_ops._CUSTOM_DVE_ROW_BASE + len(dve_ops.OPS)
    shas = {}
    for ver in ("v3", "v4"):
        try:
            shas[ver] = DveOpSpec(
                name=NAME, opcode=row, uops=lower(spec, ver=ver), rd1_en=False
            ).sha(ver)
        except Exception:
            pass
    op = dve_ops.DveOp(NAME, spec, subdim=False, uops_sha=shas)
    dve_ops.OPS.append(op)
    dve_ops.CUSTOM_DVE_SPECS[NAME] = spec
    dve_ops._SUB_OPCODE_FOR_NAME[NAME] = row
    return op


_ABS_DIFF_SCAN = _make_absdiff_scan_op()
_NC_CACHE = {}


def _build_nc():
    if "nc" in _NC_CACHE:
        return _NC_CACHE["nc"]
    nc = bacc.Bacc("TRN2", target_bir_lowering=False, debug=False,
                   num_devices=NCORES)
    xt8 = nc.declare_dram_parameter("xt8", [128, BL, NSL, S], E4M3,
                                    isOutput=False)
    w8 = nc.declare_dram_parameter("w8", [128, HT * NSL, 128], E4M3,
                                   isOutput=False)
    who = nc.declare_dram_parameter("who", [128, HT * O], F32R, isOutput=False)
    bias = nc.declare_dram_parameter("bias", [BL, O], F32, isOutput=False)
    out = nc.declare_dram_parameter("out", [BL, O], F32, isOutput=True)

    with tile.TileContext(nc) as tc:
        with (
            tc.tile_pool(name="const", bufs=1) as cpool,
            tc.tile_pool(name="xt8", bufs=XT_BUFS) as x8pool,
            tc.tile_pool(name="scan", bufs=SCAN_BUFS) as spool,
            tc.tile_pool(name="ps", bufs=PS_BUFS, space="PSUM") as pspool,
            tc.tile_pool(name="ps2", bufs=2, space="PSUM") as ps2pool,
        ):
            # weights arrive in four chunks (deps are tile-granular): the
            # first matmul group only waits for the first chunk; later chunks
            # stream in behind while earlier h-tiles compute
            WSPLITS = [(0, 2), (2, 4), (4, 8), (8, 16)]

            w8_tiles_s = []
            for i, (h0, h1) in enumerate(WSPLITS):
                w8_tiles_s.append(cpool.tile(
                    [128, (h1 - h0) * NSL, 128], E4M3, tag=f"w8p{i}",
                    name=f"w8p{i}"))
            who_sb = cpool.tile([128, HT * O], F32R, tag="who")
            bias_sb = cpool.tile([BL, O], F32, tag="bias")
            hfinal = cpool.tile([128, HT * BL], F32R, tag="hfinal")

            def w8_slice(ht, k):
                for (h0, h1), t in zip(WSPLITS, w8_tiles_s):
                    if h0 <= ht < h1:
                        o = (ht - h0) * NSL + 2 * k
                        return t[:, o:o + 2, :]
                raise AssertionError

            from bass_rust import add_dep_helper

            xt_tiles = [None] * BL
            # b=0's X lands as two split tiles so the first groups can start
            # on slots 0..3 while slots 4..7 stream in (deps tile-granular)
            xt0_parts = [
                cpool.tile([128, KDR // 2 * 2, S], E4M3, tag="x80a",
                           name="x80a"),
                cpool.tile([128, NSL - KDR // 2 * 2, S], E4M3, tag="x80b",
                           name="x80b"),
            ]
            prev_dma = nc.sync.dma_start(xt0_parts[0][:], xt8[:, 0, 0:4])

            def chain(d, sync=False):
                nonlocal prev_dma
                add_dep_helper(d.ins, prev_dma.ins, sync=sync,
                               reason="dma order")
                prev_dma = d

            # w8 part 0 rides a second queue concurrently with xt8[0]: both
            # gate the very first matmuls and are small
            d_w8p0 = nc.sync.dma_start(
                w8_tiles_s[0][:],
                w8[:, WSPLITS[0][0] * NSL:WSPLITS[0][1] * NSL, :])
            d_w8p1 = nc.sync.dma_start(
                w8_tiles_s[1][:],
                w8[:, WSPLITS[1][0] * NSL:WSPLITS[1][1] * NSL, :])
            add_dep_helper(d_w8p1.ins, d_w8p0.ins, sync=False,
                           reason="dma order")
            chain(nc.sync.dma_start(xt0_parts[1][:], xt8[:, 0, 4:NSL]))
            for i, (h0, h1) in enumerate(WSPLITS):
                if i < 2:
                    continue
                chain(nc.sync.dma_start(
                    w8_tiles_s[i][:], w8[:, h0 * NSL:h1 * NSL, :]))
            # prefetch b=1, b=2 right behind the startup chain, then the
            # epilogue constants (all of these share the 16 SDMA engines)
            for b in (1, 2):
                xt_tiles[b] = x8pool.tile([128, NSL, S], E4M3, tag="x8b",
                                          name=f"x8b{b}")
                chain(nc.sync.dma_start(xt_tiles[b][:], xt8[:, b]))
            dwho = nc.gpsimd.dma_start(who_sb[:], who[:])
            add_dep_helper(dwho.ins, prev_dma.ins, sync=True,
                           reason="defer epilogue consts")
            dbias = nc.gpsimd.dma_start(bias_sb[:], bias[:])
            add_dep_helper(dbias.ins, dwho.ins, sync=True,
                           reason="defer epilogue consts")

            def xt_chunk(b, k):
                """Moving operand [128, 2, S] for DR matmul k of batch b."""
                if b == 0:
                    if k < 2:
                        return xt0_parts[0][:, 2 * k:2 * k + 2, :]
                    return xt0_parts[1][:, 2 * (k - 2):2 * (k - 2) + 2, :]
                return xt_tiles[b][:, 2 * k:2 * k + 2, :]

            outsb = [cpool.tile([BL, 512], F32, tag=f"outsb{oc}",
                                name=f"outsb{oc}") for oc in range(O // 512)]
            ps2_tiles = [None, None]

            def emit_final(ht, ocs=(0, 1)):
                for oc in ocs:
                    if ps2_tiles[oc] is None:
                        ps2_tiles[oc] = ps2pool.tile(
                            [BL, 512], F32, tag="ps2", name=f"ps2_{oc}")
                    nc.tensor.matmul(
                        ps2_tiles[oc][:],
                        hfinal[:, ht * BL:(ht + 1) * BL],
                        who_sb[:, ht * O + oc * 512: ht * O + oc * 512 + 512],
                        start=(ht == 0),
                        stop=(ht == HT - 1),
                    )

            def ensure_xt(b):
                if b > 0 and xt_tiles[b] is None:
                    xt_tiles[b] = x8pool.tile([128, NSL, S], E4M3, tag="x8b",
                                              name=f"x8b{b}")
                    nc.sync.dma_start(xt_tiles[b][:], xt8[:, b])

            def group_tail(b, ht, ps):
                so = spool.tile([128, S], F32, tag="so")
                nc.vector._custom_dve(_ABS_DIFF_SCAN, out=so[:], in0=ps[:])
                c = ht * BL + b
                nc.scalar.copy(hfinal[:, c:c + 1], so[:, S - 1:S])

            # b=0 catch-up: while b=0's slots 4..7 and the later weight
            # chunks stream in, start the groups whose operands arrive first
            # (ht 0..3 x slots 0..3) as partial accumulation groups
            PARTIAL = [(0, 2), (1, 2), (2, 2), (3, 2)]
            pre = []
            for ht, nk in PARTIAL:
                ps = pspool.tile([128, S], F32, tag="ps")
                for k in range(nk):
                    nc.tensor.matmul(
                        ps[:], w8_slice(ht, k), xt_chunk(0, k),
                        start=(k == 0), stop=False,
                        perf_mode=mybir.MatmulPerfMode.DoubleRow,
                    )
                pre.append((ht, nk, ps))
            for ht, nk, ps in pre:
                for k in range(nk, KDR):
                    nc.tensor.matmul(
                        ps[:], w8_slice(ht, k), xt_chunk(0, k),
                        start=False, stop=(k == KDR - 1),
                        perf_mode=mybir.MatmulPerfMode.DoubleRow,
                    )
                group_tail(0, ht, ps)

            # steady state: remaining (b, h-tile) groups, batches of PS_BUFS
            groups = [(0, ht) for ht in range(len(PARTIAL), HT)]
            for b in range(1, BL):
                groups += [(b, ht) for ht in range(HT)]
            for gi in range(0, len(groups), PS_BUFS):
                batch = groups[gi:gi + PS_BUFS]
                for b, ht in batch:
                    ensure_xt(b)
                    if b + 1 < BL and ht == HT - 4:
                        ensure_xt(b + 1)
                    ps = pspool.tile([128, S], F32, tag="ps")
                    for k in range(KDR):
                        nc.tensor.matmul(
                            ps[:], w8_slice(ht, k), xt_chunk(b, k),
                            start=(k == 0), stop=(k == KDR - 1),
                            perf_mode=mybir.MatmulPerfMode.DoubleRow,
                        )
                    group_tail(b, ht, ps)
                # epilogue matmuls for b=15's h-tiles, one batch behind so
                # their scan+copy dependencies have cleared; emitted as one
                # burst per batch to avoid extra f32r<->fp8 mode switches
                for b, ht in batch:
                    if b == BL - 1 and ht > 0:
                        emit_final(ht - 1)

            # last h-tile: finish each 512-column half independently so the
            # first half's bias-add + store overlap the second half's matmul
            for oc in range(O // 512):
                emit_final(HT - 1, ocs=(oc,))
                nc.vector.tensor_tensor(
                    outsb[oc][:], ps2_tiles[oc][:],
                    bias_sb[:, oc * 512:(oc + 1) * 512], op=mybir.AluOpType.add,
                )
                nc.sync.dma_start(out[:, oc * 512:(oc + 1) * 512], outsb[oc][:])
    nc.finalize()
    _NC_CACHE["nc"] = nc
    return nc


def _q8(x):
    return np.clip(x, -240, 240).astype(E4NP).astype(np.float32)


def _shape_X(Xs):
    """First-order noise shaping along t (axis 0) of pre-scaled X [S, B, I].

    Bounds every time-window sum of quantization error to ~1 ulp so the
    |.|-recurrence (a running sum once h has grown) sees almost no
    accumulated X quantization drift."""
    out = np.empty_like(Xs)
    e = np.zeros(Xs.shape[1:], np.float32)
    for t in range(S):
        v = Xs[t] + e
        q = _q8(v)
        e = v - q
        out[t] = q
    return out


def _ulp_e4(x):
    ax = np.maximum(np.abs(x), 2.0 ** -6)
    return (2.0 ** (np.floor(np.log2(ax)) - 3)).astype(np.float32)


def _fix_W(Wt, basis, niter=FLIP_ITERS, lam=FLIP_LAM):
    """GPTQ-style rounding optimization: toggle per-element rounding
    direction of q8(Wt) so the quant error is near-orthogonal to `basis`
    ([K, I] per-batch X prefix-sum windows), cancelling the coherent
    W-error term of the scan. Greedy 1-opt, one toggle per row per iter."""
    W8 = _q8(Wt)
    dW0 = (W8 - Wt).astype(np.float32)
    dW = dW0.copy()
    c = dW @ basis.T
    step = _ulp_e4(W8)
    colsq = (basis ** 2).sum(axis=0)
    nz = colsq.mean()
    flipped = np.zeros(dW.shape, bool)
    rows = np.arange(Wt.shape[0])
    for _ in range(niter):
        delta = np.where(flipped, dW0 - dW, -np.sign(dW0 + 1e-30) * step)
        cb = c @ basis
        score = 2 * delta * cb + (delta ** 2) * colsq[None, :] \
            + lam * nz * (delta ** 2 + 2 * dW * delta)
        j = np.argmin(score, axis=1)
        apply = score[rows, j] < -1e-6
        if not apply.any():
            break
        r, jj = rows[apply], j[apply]
        c[r] += delta[r, jj, None] * basis[:, jj].T
        dW[r, jj] += delta[r, jj]
        flipped[r, jj] = ~flipped[r, jj]
    return _q8(Wt + dW)


def _prep_inputs(X, W_ih, HH, W_ho, b_ho):
    """Host-side sharding + relayout + quantization refinement.

    Returns list of per-core input maps."""
    X = np.asarray(X, dtype=np.float32)
    W_ih = np.asarray(W_ih, dtype=np.float32)
    HH = np.asarray(HH, dtype=np.float32)
    W_ho = np.asarray(W_ho, dtype=np.float32)
    b_ho = np.asarray(b_ho, dtype=np.float32)

    # Fold sign(HH) into W_ih rows: |xw + HH*h| = |sgn*xw + |HH|*h| for h>=0.
    if not np.all(HH == 1.0):
        sgn = np.where(HH < 0, -1.0, 1.0).astype(np.float32)
        W_ih = W_ih * sgn[:, None]
        HH = np.abs(HH)
        if not np.allclose(HH, 1.0):
            raise NotImplementedError(
                "general |HH| != 1 recurrence not implemented in this kernel"
            )

    Wn = (-W_ih * SW).astype(np.float32)  # negated (scan computes |h-(-xw)|)
    X8 = _shape_X(X * SX)                 # [S, B, I] f32-valued fp8 numbers

    # sqrt-spaced prefix-sum windows (reflections of the scan cluster early)
    grid = sorted(set(max(1, round(((j + 1) / NWIN) ** 2 * S))
                      for j in range(NWIN)))
    cs = np.cumsum(X8, axis=0)
    # who[p, ht, o] = W_ho[o, ht*128+p] / (SX*SW)  (descale folded in)
    who = np.ascontiguousarray(
        (W_ho / (SX * SW)).reshape(O, HT, 128).transpose(2, 1, 0)
    ).reshape(128, -1)
    bias = np.ascontiguousarray(np.broadcast_to(b_ho, (BL, O)))

    in_maps = []
    for c in range(NCORES):
        bsl = slice(c * BL, (c + 1) * BL)
        basis = np.concatenate([cs[g - 1, bsl] for g in grid], axis=0)
        W8c = _fix_W(Wn, basis)
        # w8[p, ht*8+cc, hh] = W8c[ht*128+hh, cc*128+p]
        w8 = np.ascontiguousarray(
            W8c.reshape(HT, 128, NSL, 128).transpose(3, 0, 2, 1)
        ).reshape(128, HT * NSL, 128).astype(E4NP)
        # xt8[p, b, cc, s] = X8[s, b_global, cc*128+p]
        xt8 = np.ascontiguousarray(
            X8[:, bsl].reshape(S, BL, NSL, 128).transpose(3, 1, 2, 0)
        ).astype(E4NP)
        in_maps.append({"xt8": xt8, "w8": w8, "who": who, "bias": bias})
    return in_maps


def _run(in_maps, **kwargs):
    nc = _build_nc()
    return run_bass_kernel_spmd(nc, in_maps, core_ids=list(range(NCORES)),
                                **kwargs)


def kernel(X, W_ih, HH, W_ho, b_ho):
    in_maps = _prep_inputs(X, W_ih, HH, W_ho, b_ho)
    res = _run(in_maps)
    return np.concatenate([res.results[c]["out"] for c in range(NCORES)],
                          axis=0)
